# revision 2
# baseline (speedup 1.0000x reference)
"""DualAttention Trainium2 kernel v2.

Problem: x:[2,64,20,20,20]; three separable 1-D convs produce q0 (H-axis),
k0 (D-axis), v (W-axis), each [B,C,N] with N=8000; scores = k0^T q0 [B,N,N];
softmax over the key axis i (axis 1); out = v @ attn, reshaped back.

Sharding: 8 cores = 2 batches x 4 query-column slices of 2000. Each core
computes full k0/v (cheap convs) and its q0 slice, then a flash-style
scores->exp->accumulate loop. No collectives.

v2 changes vs v1 baseline:
  - all conv inputs/weights in bf16 (fp32 matmul is 1/4 rate on PE; conv
    outputs were stored bf16 anyway). Halves input DMA too.
  - k bias dropped (adds a per-j constant to scores -> softmax-invariant);
    q/v biases folded into the conv matmuls via a ones-row in the center
    input slab (K=65) and a bias row in the weights. No bias-add DVE ops.
  - v transposed via the DMA xbar engine (dma_start_transpose) into a
    contiguous [128, 63*64] buffer + one strided DVE copy into the
    65-stride [vT | 1] layout, instead of 63 PE-transposes.
  - exp split across two engines: even i-tiles on ScalarE (exact exp),
    odd i-tiles on VectorE via a Schraudolph bit-trick
    (uint16(round(s*128/ln2 + B)) bitcast to bf16), which halves the
    137us ScalarE exp wall. B is calibrated end-to-end (mean-ratio ~1 so
    DVE tiles carry no systematic attention tilt vs ACT tiles).
"""

import numpy as np
import ml_dtypes

B, C, D, H, W = 2, 64, 20, 20, 20
N = D * H * W            # 8000
NCORES = 8
CPB = 4                  # cores per batch
MS = N // CPB            # 2000 query columns per core
ITILE = 128
NFULL = N // ITILE       # 62 full i-tiles
TAILW = N - NFULL * ITILE  # 64
NT = NFULL + 1           # 63
NV = NT * ITILE          # 8064 (v length padded to full tiles)
M_PASSES = [(0, 1024), (1024, 976)]
EXP_BIAS = -8.0
LN2 = 0.6931471805599453
SCH_S = 128.0 / LN2
SCH_B0 = 16250.49        # Schraudolph offset, tuned end-to-end
SCH_B = SCH_B0 + EXP_BIAS * SCH_S
DVE_TILES = frozenset(range(1, NFULL, 2)) | {NFULL}  # odd i-tiles + tail on VectorE

_CACHE = {}


def _build_bass(reps=1, bench_loop=0):
    import contextlib
    import concourse.tile as tile
    import concourse.mybir as mybir
    from concourse import bacc

    F32 = mybir.dt.float32
    BF16 = mybir.dt.bfloat16
    U16 = mybir.dt.uint16
    EXP = mybir.ActivationFunctionType.Exp
    LN = mybir.ActivationFunctionType.Ln
    ADD = mybir.AluOpType.add
    MULT = mybir.AluOpType.mult

    nc = bacc.Bacc("TRN2", target_bir_lowering=False, debug=False,
                   num_devices=NCORES)

    xin_d = nc.dram_tensor("xin", [65, N], BF16, kind="ExternalInput")
    xD_d = nc.dram_tensor("xD", [128, N], BF16, kind="ExternalInput")
    xW_d = nc.dram_tensor("xW", [128, N], BF16, kind="ExternalInput")
    xH_d = nc.dram_tensor("xH", [128, MS], BF16, kind="ExternalInput")
    xq_d = nc.dram_tensor("xq", [65, MS], BF16, kind="ExternalInput")
    wp_d = nc.dram_tensor("wp", [128, 384], BF16, kind="ExternalInput")
    out_d = nc.dram_tensor("out", [64, MS], F32, kind="ExternalOutput")

    PIECE = 2048  # input DMA piece size for pipelining

    with tile.TileContext(nc) as tc:
        loop_cm = (tc.For_i(0, bench_loop, 1,
                            hint_engines=(mybir.EngineType.PE,),
                            staggered_reset=True)
                   if bench_loop else contextlib.nullcontext())
        with loop_cm:
         for _rep in range(reps):
            with tc.tile_pool(name="persist", bufs=1) as per, \
                 tc.tile_pool(name="mps", bufs=3, space="PSUM") as mp, \
                 tc.tile_pool(name="ops", bufs=1, space="PSUM") as opp, \
                 tc.tile_pool(name="eps", bufs=10) as ep, \
                 tc.tile_pool(name="fin", bufs=1) as fin, \
                 tc.tile_pool(name="prep", bufs=1) as pr:

                w_sb = per.tile([128, 384], BF16, name="w_sb")
                nc.gpsimd.dma_start(w_sb[:, :], wp_d.ap())
                k0_sb = per.tile([128, N], BF16, name="k0_sb")
                q0_sb = per.tile([128, MS], BF16, name="q0_sb")
                vT_sb = per.tile([128, NT * 65], BF16, name="vT_sb")
                out_sb = per.tile([64, MS], F32, name="out_sb")
                expb = per.tile([128, 1], F32, name="expb")

                nc.vector.memset(expb[:, :], EXP_BIAS)
                nc.gpsimd.memset(vT_sb[:, :], 1.0)
                vT_view = vT_sb[:, :].rearrange("p (t c) -> p t c", c=65)

                # tiny warmup exp: hosts the ACT table load at body start,
                # where it hides under the input-DMA latency
                warm = per.tile([1, 2], F32, name="warm")
                nc.vector.memset(warm[:, :], 1.0)
                nc.scalar.activation(warm[:, :], warm[:, :], EXP)

                def conv_pair(dst, wS, w0, krows, s0_sb, sS_sb, c0, cw2, name,
                              evac):
                    # up to 1024 cols: two <=512-col matmul chunks into one
                    # psum slot, evacuated with a single copy
                    ps = mp.tile([128, 1024], F32, tag="s", name=name)
                    for (d0, dw) in [(0, min(512, cw2)), (512, cw2 - 512)]:
                        if dw <= 0:
                            break
                        nc.tensor.matmul(ps[0:64, d0:d0 + dw],
                                         lhsT=w_sb[:, wS:wS + 64],
                                         rhs=sS_sb[:, c0 + d0:c0 + d0 + dw],
                                         start=True, stop=False)
                        nc.tensor.matmul(ps[0:64, d0:d0 + dw],
                                         lhsT=w_sb[0:krows, w0:w0 + 64],
                                         rhs=s0_sb[0:krows, c0 + d0:c0 + d0 + dw],
                                         start=False, stop=True)
                    if evac == "act":
                        nc.scalar.copy(dst[0:64, c0:c0 + cw2], ps[0:64, 0:cw2])
                    else:
                        nc.vector.tensor_copy(out=dst[0:64, c0:c0 + cw2],
                                              in_=ps[0:64, 0:cw2])

                # --- q conv first (small inputs, unblocks scores) ---
                xH_sb = pr.tile([128, MS], BF16, tag="xh", name="xH_sb")
                xq_sb = pr.tile([65, MS], BF16, tag="xq", name="xq_sb")
                for p0 in (0, 1024):
                    pw = min(1024, MS - p0)
                    nc.scalar.dma_start(xH_sb[:, p0:p0 + pw], xH_d.ap()[:, p0:p0 + pw])
                    nc.scalar.dma_start(xq_sb[:, p0:p0 + pw], xq_d.ap()[:, p0:p0 + pw])
                for c0 in range(0, MS, 1024):
                    cw2 = min(1024, MS - c0)
                    conv_pair(q0_sb, 128, 192, 65, xq_sb, xH_sb,
                              c0, cw2, f"q{c0}", "act")
                    nc.scalar.dma_start(q0_sb[64:128, c0:c0 + cw2],
                                        q0_sb[0:64, c0:c0 + cw2])

                # --- k conv, pipelined with pieced input DMA; per-chunk dup ---
                xin_sb = pr.tile([65, N], BF16, tag="xin", name="xin_sb")
                xD_sb = pr.tile([128, N], BF16, tag="xd", name="xD_sb")
                for p0 in range(0, N, PIECE):
                    pw = min(PIECE, N - p0)
                    nc.sync.dma_start(xD_sb[:, p0:p0 + pw], xD_d.ap()[:, p0:p0 + pw])
                    nc.gpsimd.dma_start(xin_sb[:, p0:p0 + pw],
                                        xin_d.ap()[:, p0:p0 + pw])
                for c0 in range(0, N, 1024):
                    cw2 = min(1024, N - c0)
                    conv_pair(k0_sb, 0, 64, 64, xin_sb, xD_sb, c0, cw2,
                              f"k{c0}", "act")
                    nc.gpsimd.dma_start(k0_sb[64:128, c0:c0 + cw2],
                                        k0_sb[0:64, c0:c0 + cw2])

                # xW on the scalar queue (idle after xH/xq), parallel to xD
                xW_sb = pr.tile([128, N], BF16, tag="xw", name="xW_sb")
                for p0 in range(0, N, PIECE):
                    pw = min(PIECE, N - p0)
                    nc.scalar.dma_start(xW_sb[:, p0:p0 + pw], xW_d.ap()[:, p0:p0 + pw])

                def scores_exp(m0, mw, t, rows, name):
                    s = mp.tile([128, mw], F32, tag="s", name=f"s{name}")
                    h = (t % 2) * 64
                    for (c0, cw) in [(0, 512), (512, mw - 512)]:
                        nc.tensor.matmul(
                            s[0:rows, c0:c0 + cw],
                            lhsT=k0_sb[h:h + 64, t * ITILE:t * ITILE + rows],
                            rhs=q0_sb[h:h + 64, m0 + c0:m0 + c0 + cw],
                            start=True, stop=True)
                    e = ep.tile([128, mw], BF16, tag="e", name=f"e{name}")
                    if t in DVE_TILES:
                        nc.vector.tensor_scalar(
                            out=e[0:rows, :].bitcast(U16), in0=s[0:rows, :],
                            scalar1=SCH_S, scalar2=SCH_B, op0=MULT, op1=ADD)
                    else:
                        nc.scalar.activation(e[0:rows, :], s[0:rows, :], EXP,
                                             bias=expb[0:rows, :])
                    return e

                def out_mms(po, mw, t, rows, e, first, last):
                    for (c0, cw) in [(0, 512), (512, mw - 512)]:
                        nc.tensor.matmul(po[:, c0:c0 + cw],
                                         lhsT=vT_view[0:rows, t, :],
                                         rhs=e[0:rows, c0:c0 + cw],
                                         start=first, stop=last)

                # hoist the first pairs of pass 1: their scores+exp run
                # while the v conv occupies the PE, so ACT/DVE stay busy
                # through the prep tail
                HOIST = 3
                hoisted = []
                for p in range(HOIST):
                    for t in (2 * p, 2 * p + 1):
                        hoisted.append(
                            (t, scores_exp(0, 1024, t, ITILE, f"h{t}")))

                # --- v conv -> v_sb, then xbar-transpose into vT ---
                v_sb = pr.tile([64, NV], BF16, tag="v", name="v_sb")
                nc.gpsimd.memset(v_sb[:, N:NV], 0.0)
                for c0 in range(0, N, 1024):
                    conv_pair(v_sb, 256, 320, 65, xin_sb, xW_sb,
                              c0, min(1024, N - c0), f"v{c0}", "dve")
                vT2_sb = pr.tile([128, NT * 64], BF16, tag="vt2", name="vT2_sb")
                HALF_T = 32
                for (t0, t1) in [(0, HALF_T), (HALF_T, NT)]:
                    nc.sync.dma_start_transpose(
                        vT2_sb[:, t0 * 64:t1 * 64].rearrange(
                            "p (t c) -> p t c", c=64),
                        v_sb[:, t0 * ITILE:t1 * ITILE])
                    nc.vector.tensor_copy(
                        out=vT_view[:, t0:t1, 0:64],
                        in_=vT2_sb[:, t0 * 64:t1 * 64].rearrange(
                            "p (t c) -> p t c", c=64))

                # --- main attention loop, software-pipelined so PE never
                #     waits on the current iteration's exp ---
                for (m0, mw) in M_PASSES:
                    po = opp.tile([65, mw], F32, tag="po", name=f"po{m0}")
                    pend = []
                    if m0 == 0:
                        emitted_first = True
                        for (t, e) in hoisted:
                            out_mms(po, mw, t, ITILE, e,
                                    t == hoisted[0][0], False)
                        p_start = HOIST
                    else:
                        emitted_first = False
                        p_start = 0
                    for p in range(p_start, NFULL // 2):
                        tA, tB = 2 * p, 2 * p + 1
                        eA = scores_exp(m0, mw, tA, ITILE, f"A{m0}_{p}")
                        eB = scores_exp(m0, mw, tB, ITILE, f"B{m0}_{p}")
                        pend.append((tA, eA))
                        pend.append((tB, eB))
                        while len(pend) > 4:
                            t0, e0 = pend.pop(0)
                            out_mms(po, mw, t0, ITILE, e0,
                                    not emitted_first, False)
                            emitted_first = True
                    eT = scores_exp(m0, mw, NFULL, TAILW, f"T{m0}")
                    for (t0, e0) in pend:
                        out_mms(po, mw, t0, ITILE, e0, not emitted_first, False)
                        emitted_first = True
                    pend = []
                    out_mms(po, mw, NFULL, TAILW, eT, False, True)

                    # normalize: out = numerator * (1 / denominator-row),
                    # chunked so each 512-col chain (reciprocal -> broadcast
                    # -> multiply -> DMA) overlaps the next
                    rc = fin.tile([1, mw], F32, tag="rc", name=f"rc{m0}")
                    bc = fin.tile([64, mw], F32, tag="bc", name=f"bc{m0}")
                    for (c0, cw) in [(0, 512), (512, mw - 512)]:
                        nc.vector.reciprocal(
                            rc[:, c0:c0 + cw], po[64:65, c0:c0 + cw])
                        nc.gpsimd.partition_broadcast(
                            bc[:, c0:c0 + cw], rc[:, c0:c0 + cw], channels=64)
                        nc.vector.tensor_tensor(
                            out=out_sb[0:64, m0 + c0:m0 + c0 + cw],
                            in0=po[0:64, c0:c0 + cw], in1=bc[:, c0:c0 + cw],
                            op=MULT)
                        nc.sync.dma_start(out_d.ap()[:, m0 + c0:m0 + c0 + cw],
                                          out_sb[:, m0 + c0:m0 + c0 + cw])
    nc.compile()
    return nc


def _shifted(xb):
    """xb [C, D, H, W] -> dict of zero-padded unit shifts, flattened [C, N]."""
    z = np.zeros_like(xb)
    sDp = z.copy(); sDp[:, :-1] = xb[:, 1:]
    sDm = z.copy(); sDm[:, 1:] = xb[:, :-1]
    sHp = z.copy(); sHp[:, :, :-1] = xb[:, :, 1:]
    sHm = z.copy(); sHm[:, :, 1:] = xb[:, :, :-1]
    sWp = z.copy(); sWp[..., :-1] = xb[..., 1:]
    sWm = z.copy(); sWm[..., 1:] = xb[..., :-1]
    f = lambda a: a.reshape(C, N)
    return {k: f(v) for k, v in dict(Dp=sDp, Dm=sDm, Hp=sHp, Hm=sHm,
                                     Wp=sWp, Wm=sWm).items()}


def _bf16(a):
    return np.ascontiguousarray(np.asarray(a, np.float32).astype(ml_dtypes.bfloat16))


def _pack_weights(q_w, k_w, v_w, q_b, v_b):
    """[128, 384] bf16 pack. Col blocks of 64 (each [in-dim rows, out cols]):
    0: k +/- taps stacked (rows 0:64 = plus tap, 64:128 = minus);
    64: k center (rows 0:64); 128/192: q ditto with q_b in row 64 of center;
    256/320: v ditto with v_b in row 64 of center."""
    kw = k_w[:, :, :, 0, 0]   # [O, I, 3] taps along D
    qw = q_w[:, :, 0, :, 0]   # taps along H
    vw = v_w[:, :, 0, 0, :]   # taps along W
    wp = np.zeros((128, 384), np.float32)

    def put(col, w3, b=None):
        wp[0:64, col:col + 64] = np.ascontiguousarray(w3[:, :, 2].T)
        wp[64:128, col:col + 64] = np.ascontiguousarray(w3[:, :, 0].T)
        wp[0:64, col + 64:col + 128] = np.ascontiguousarray(w3[:, :, 1].T)
        if b is not None:
            wp[64, col + 64:col + 128] = b

    put(0, kw)
    put(128, qw, q_b)
    put(256, vw, v_b)
    return _bf16(wp)


def make_in_maps(x, q_w, q_b, k_w, k_b, v_w, v_b):
    x = np.asarray(x, np.float32)
    wp = _pack_weights(np.asarray(q_w, np.float32),
                       np.asarray(k_w, np.float32),
                       np.asarray(v_w, np.float32),
                       np.asarray(q_b, np.float32),
                       np.asarray(v_b, np.float32))
    ones_n = np.ones((1, N), np.float32)
    in_maps = []
    per_batch = []
    for b in range(B):
        xb = x[b]
        x2 = xb.reshape(C, N)
        sh = _shifted(xb)
        xin65 = _bf16(np.vstack([x2, ones_n]))
        xD = _bf16(np.vstack([sh["Dp"], sh["Dm"]]))
        xW = _bf16(np.vstack([sh["Wp"], sh["Wm"]]))
        per_batch.append((x2, sh, xin65, xD, xW))
    for g in range(NCORES):
        b, s = g // CPB, g % CPB
        x2, sh, xin65, xD, xW = per_batch[b]
        off = s * MS
        in_maps.append({
            "xin": xin65,
            "xD": xD,
            "xW": xW,
            "xH": _bf16(np.vstack([sh["Hp"][:, off:off + MS],
                                   sh["Hm"][:, off:off + MS]])),
            "xq": _bf16(np.vstack([x2[:, off:off + MS], ones_n[:, :MS]])),
            "wp": wp,
        })
    return in_maps


def kernel(x, q_w, q_b, k_w, k_b, v_w, v_b, trace=False):
    from concourse.bass_utils import run_bass_kernel_spmd
    if "nc" not in _CACHE:
        _CACHE["nc"] = _build_bass()
    nc = _CACHE["nc"]
    in_maps = make_in_maps(x, q_w, q_b, k_w, k_b, v_w, v_b)
    res = run_bass_kernel_spmd(nc, in_maps, core_ids=list(range(NCORES)),
                               trace=trace)
    _CACHE["last_result"] = res
    out = np.empty((B, C, N), np.float32)
    for g in range(NCORES):
        b, s = g // CPB, g % CPB
        out[b, :, s * MS:(s + 1) * MS] = res.results[g]["out"]
    return out.reshape(B, C, D, H, W)
